# revision 18
# baseline (speedup 1.0000x reference)
"""Single-head causal attention (B=8, T=2048, E=1024, H=64) on 8 trn2
cores, data-parallel over batch (core b handles batch element b).

Precision layout (validated vs gate rel_err < 2e-2; measures 8.1e-3):
  - q and v projected in fp16 via one shared stationary [Wq|Wv] (the v
    projection rides free in the 128-wide weight block),
  - k projected in fp8e4m3 DoubleRow (host pair-packs x and Wk along the
    contraction dim: E = 256g + 2p + k on partition p, pair k -> 256-deep
    contraction per matmul, half the matmul count),
  - scoresT, exp and PV all fp16 (fp8 there is LDWEIGHTS-bound and the
    v path cannot tolerate fp8 quantization at the max-error tail).

Host marshaling: x is cast/transposed/packed on host (fp16 xT + fp8
pair-packed copy), weights pre-swizzled partition-major, identity and
causal-mask constants shipped as inputs so the device has no const loads.

Device schedule per core:
  proj (overlapped with pipelined x DMAs; qv j-outer / k g-outer into all
  8 psum banks so the PE starts on the first arriving tile):
    pqv[c] += wqv_j^T @ x16_j[:, chunk]      (fp16)
    pk[c]  += wk8_g^T @ x8_g[:, :, chunk]    (fp8 DoubleRow)
    evacuate q,k -> fp16 partitions 0:63 (directly usable by scores),
    v -> rows 64:127; v1[s, 65] = PE-transpose(v) + ones column (the
    ones column makes PV accumulate the softmax denominator for free)
  attention (j = key block 0..15, PV lagging one block):
    scoresT[s,t] = k16_j^T @ q16 -> exp((1/8)x) on ACT -> wT_j fp16
    diag tri-mask on DVE; outT[65,t] += v1_j^T @ wT_j (psum bank/512-chunk)
    as chunk c completes: PE-transpose outT, reciprocal-normalize
    (DVE/ACT split), store f32
"""

import numpy as np
import ml_dtypes

import concourse.bass as bass
import concourse.mybir as mybir
from concourse.tile import TileContext
from concourse.bass_utils import run_bass_kernel_spmd

B, T, E, H = 8, 2048, 1024, 64
NT = T // 128   # 16 key/row blocks
NE = E // 128   # 8 fp16 contraction blocks
NG = E // 256   # 4 fp8 pair-contraction groups
NC = T // 512   # 4 column chunks
F16 = mybir.dt.float16
F32 = mybir.dt.float32
F8 = mybir.dt.float8e4
NP8 = ml_dtypes.float8_e4m3
SCALE = float(H) ** -0.5
DR = mybir.MatmulPerfMode.DoubleRow


def _split_excess_waits(nc: bass.Bass, cap: int = 1) -> int:
    n_split = 0
    for f in nc.m.functions:
        for bb in f.blocks:
            insts = list(bb.instructions)
            out = []
            dirty = False
            for inst in insts:
                si = inst.sync_info
                waits = list(si.on_wait) if si and si.on_wait else []
                if len(waits) > cap:
                    si.on_wait = waits[:cap]
                    for w in waits[cap:]:
                        nop = mybir.InstNoOp(
                            name=f"I-waitsplit-{n_split}", ins=[], outs=[]
                        )
                        nop.engine = inst.engine
                        nop.sync_info = mybir.SyncInfo(on_wait=[w], on_update=[])
                        out.append(nop)
                        n_split += 1
                    dirty = True
                out.append(inst)
            if dirty:
                bb.instructions = out
    return n_split


def build_nc(split_waits: bool = True) -> bass.Bass:
    nc = bass.Bass()
    x16 = nc.dram_tensor("x16", [E, T], F16, kind="ExternalInput")
    x8 = nc.dram_tensor("x8", [128, NG * 2 * T], F8, kind="ExternalInput")
    wqv16 = nc.dram_tensor("wqv16", [128, NE * 128], F16, kind="ExternalInput")
    wk8 = nc.dram_tensor("wk8", [128, NG * 2 * H], F8, kind="ExternalInput")
    eye64v_d = nc.dram_tensor("eye64v", [128, 64], F16, kind="ExternalInput")
    eye32_d = nc.dram_tensor("eye32", [128, 128], F32, kind="ExternalInput")
    tri_d = nc.dram_tensor("tri", [128, 128], F16, kind="ExternalInput")
    out = nc.dram_tensor("out", [T, H], F32, kind="ExternalOutput")
    x16_ap, x8_ap, out_ap = x16.ap(), x8.ap(), out.ap()

    with TileContext(nc) as tc:
        with (
            tc.tile_pool(name="const", bufs=1) as cpool,
            tc.tile_pool(name="wts", bufs=1) as wpool,
            tc.tile_pool(name="xt", bufs=8) as xtpool,
            tc.tile_pool(name="x8t", bufs=4) as x8pool,
            tc.tile_pool(name="qkv", bufs=1) as qkvpool,
            tc.tile_pool(name="wTp", bufs=4) as wtpool,
            tc.tile_pool(name="fin", bufs=2) as finpool,
        ):
            wqv_t = wpool.tile([128, NE * 128], F16, tag="wqv")
            nc.scalar.dma_start(wqv_t[:], wqv16.ap())
            wk_t = wpool.tile([128, NG * 2 * H], F8, tag="wk")
            nc.scalar.dma_start(wk_t[:], wk8.ap())

            xts = [
                xtpool.tile([128, T], F16, tag="xt", name=f"xt{j}")
                for j in range(NE)
            ]
            x8ts = [
                x8pool.tile([128, 2 * T], F8, tag="x8t", name=f"x8t{g}")
                for g in range(NG)
            ]
            x8_r = x8_ap.rearrange("p (g k t) -> p g k t", g=NG, k=2)
            # interleave x16/x8 loads in MM consumption order
            for j in range(NE):
                for h in range(2):
                    sl = slice(1024 * h, 1024 * h + 1024)
                    nc.sync.dma_start(
                        xts[j][:, sl], x16_ap[128 * j : 128 * j + 128, sl]
                    )
                if j % 2 == 1:
                    g = j // 2
                    xg = x8ts[g][:].rearrange("p (k t) -> p k t", k=2)
                    for h in range(2):
                        sl = slice(1024 * h, 1024 * h + 1024)
                        nc.sync.dma_start(xg[:, :, sl], x8_r[:, g, :, sl])

            eye64v = cpool.tile([128, 64], F16, tag="eye64v")
            nc.sync.dma_start(eye64v[:], eye64v_d.ap())
            eye32 = cpool.tile([128, 128], F32, tag="eye32")
            nc.sync.dma_start(eye32[:], eye32_d.ap())
            tri = cpool.tile([128, 128], F16, tag="tri")
            nc.sync.dma_start(tri[:], tri_d.ap())
            zb = cpool.tile([128, 1], F32, tag="zb")
            nc.gpsimd.memset(zb[:], 0.0)
            expwarm = cpool.tile([128, 1], F16, tag="expwarm")
            nc.scalar.activation(
                expwarm[:], zb[:], mybir.ActivationFunctionType.Exp,
                bias=zb[:, 0:1], scale=1.0,
            )

            q16 = qkvpool.tile([64, T], F16, tag="q16")
            k16 = qkvpool.tile([64, T], F16, tag="k16")
            vTT = qkvpool.tile([128, T], F16, tag="vTT")
            v1a = qkvpool.tile([128, NT * 33], F16, tag="v1a")
            v1b = qkvpool.tile([128, NT * 32], F16, tag="v1b")
            nc.gpsimd.memset(
                v1a[:].rearrange("p (i c) -> p i c", c=33)[:, :, 32:33], 1.0
            )

            wk_r = wk_t[:].rearrange("p (g k m) -> p g k m", g=NG, k=2)

            # ---------- projections ----------
            with tc.tile_pool(name="ps12", bufs=1, space="PSUM") as ps12:
                pqv = ps12.tile([128, 2048], F32, tag="pqv", bufs=1)
                pk = ps12.tile([64, 2048], F32, tag="pk", bufs=1)
                # pqv chunks (fp16, q rows 0:64 / v rows 64:128); pk (fp8 DR)
                for j in range(NE):
                    for half in range(2):
                        for c in (2 * half, 2 * half + 1):
                            nc.tensor.matmul(
                                pqv[:, 512 * c : 512 * c + 512],
                                wqv_t[:, 128 * j : 128 * j + 128],
                                xts[j][:, 512 * c : 512 * c + 512],
                                start=(j == 0), stop=(j == NE - 1),
                            )
                    if j % 2 == 1:
                        g = j // 2
                        xg = x8ts[g][:].rearrange("p (k t) -> p k t", k=2)
                        for half in range(2):
                            for c in (2 * half, 2 * half + 1):
                                nc.tensor.matmul(
                                    pk[:, 512 * c : 512 * c + 512],
                                    wk_r[:, g],
                                    xg[:, :, 512 * c : 512 * c + 512],
                                    start=(g == 0), stop=(g == NG - 1),
                                    perf_mode=DR,
                                )

                # evacuate: q,k -> fp16 at partitions 0:63, v -> rows 64:127
                # (narrow first slices so scores j=0 starts ASAP)
                s0a, s0b = slice(0, 512), slice(512, 2048)
                nc.vector.tensor_copy(q16[:, s0a], pqv[0:64, s0a])
                nc.scalar.copy(k16[:, s0a], pk[:, s0a])
                nc.vector.tensor_copy(q16[:, s0b], pqv[0:64, s0b])
                nc.scalar.copy(k16[:, s0b], pk[:, s0b])
                nc.vector.tensor_copy(
                    vTT[64:128, 0:1024], pqv[64:128, 0:1024]
                )
                nc.scalar.copy(vTT[64:128, 1024:2048], pqv[64:128, 1024:2048])
                # PE keep-warm burners: dead matmuls into pk bank 0 (already
                # evacuated) so the HAM activity window never sees an idle
                # bridge and the PE clock stays at 2.4GHz into attention
                for _ in range(3):
                    nc.tensor.matmul(
                        pk[0:64, 0:512], wqv_t[:, 0:64], xts[0][:, 0:512],
                        start=True, stop=True,
                    )

                # v1 = transpose(v) blocks + ones column
                for g in range(2):
                    tp = ps12.tile(
                        [128, 512], F32, tag="pp", bufs=8, name=f"tp{g}"
                    )
                    for i in range(8):
                        blk = 8 * g + i
                        nc.tensor.matmul(
                            tp[:, 64 * i : 64 * i + 64],
                            vTT[64:128, 128 * blk : 128 * blk + 128],
                            eye64v[64:128, :],
                            start=True, stop=True,
                        )
                    tp_r = tp[:].rearrange("p (i c) -> p i c", c=64)
                    nc.vector.tensor_copy(
                        v1a[:, 264 * g : 264 * g + 264].rearrange(
                            "p (i c) -> p i c", c=33
                        )[:, :, 0:32],
                        tp_r[:, :, 0:32],
                    )
                    nc.vector.tensor_copy(
                        v1b[:, 256 * g : 256 * g + 256].rearrange(
                            "p (i c) -> p i c", c=32
                        ),
                        tp_r[:, :, 32:64],
                    )

            # ---------- attention: scores, exp, PV, finish ----------
            with tc.tile_pool(name="ps3", bufs=1, space="PSUM") as ps3:
                otps = [
                    ps3.tile([128, 512], F32, tag="ot", bufs=4, name=f"ot{c}")
                    for c in range(NC)
                ]
                wTs = {}

                def emit_pv(j):
                    s0 = 128 * j
                    for c in range(s0 // 512, NC):
                        t0 = max(512 * c, s0)
                        t1 = 512 * c + 512
                        wsl = wTs[j][:, t0 - s0 : t1 - s0]
                        nc.tensor.matmul(
                            otps[c][0:33, t0 - 512 * c : 512],
                            v1a[:, 33 * j : 33 * j + 33],
                            wsl,
                            start=(j == 0), stop=(j == 4 * c + 3),
                            tile_position=(0, 0), skip_group_check=True,
                        )
                        nc.tensor.matmul(
                            otps[c][64:96, t0 - 512 * c : 512],
                            v1b[:, 32 * j : 32 * j + 32],
                            wsl,
                            start=(j == 0), stop=(j == 4 * c + 3),
                            tile_position=(0, 64), skip_group_check=True,
                        )

                def emit_finish(c):
                    oc = finpool.tile(
                        [128, 512], F32, tag="oc", bufs=2, name=f"oc{c}"
                    )
                    nc.vector.tensor_copy(oc[0:33, :], otps[c][0:33, :])
                    nc.scalar.copy(oc[64:96, :], otps[c][64:96, :])
                    ft = ps3.tile(
                        [128, 260], F32, tag="ot", bufs=4, name=f"ft{c}"
                    )
                    for i in range(4):
                        nc.tensor.transpose(
                            ft[:, 65 * i : 65 * i + 33],
                            oc[0:33, 128 * i : 128 * i + 128],
                            eye32[0:33, 0:33],
                        )
                        nc.tensor.transpose(
                            ft[:, 65 * i + 33 : 65 * i + 65],
                            oc[64:96, 128 * i : 128 * i + 128],
                            eye32[64:96, 64:96],
                        )
                    rcp = finpool.tile(
                        [128, 4], F32, tag="rcp", bufs=2, name=f"rcp{c}"
                    )
                    nc.vector.reciprocal(
                        rcp[:],
                        ft[:].rearrange("p (i c) -> p i c", c=65)[:, :, 32:33],
                    )
                    ob = finpool.tile(
                        [128, 256], F32, tag="ob", bufs=2, name=f"ob{c}"
                    )
                    for i in range(4):
                        if i % 2 == 0:
                            nc.vector.tensor_scalar_mul(
                                ob[:, 64 * i : 64 * i + 32],
                                ft[:, 65 * i : 65 * i + 32],
                                rcp[:, i : i + 1],
                            )
                            nc.scalar.mul(
                                ob[:, 64 * i + 32 : 64 * i + 64],
                                ft[:, 65 * i + 33 : 65 * i + 65],
                                rcp[:, i : i + 1],
                            )
                        else:
                            nc.scalar.mul(
                                ob[:, 64 * i : 64 * i + 32],
                                ft[:, 65 * i : 65 * i + 32],
                                rcp[:, i : i + 1],
                            )
                            nc.vector.tensor_scalar_mul(
                                ob[:, 64 * i + 32 : 64 * i + 64],
                                ft[:, 65 * i + 33 : 65 * i + 65],
                                rcp[:, i : i + 1],
                            )
                    nc.sync.dma_start(
                        out_ap[512 * c : 512 * c + 512, :].rearrange(
                            "(i p) h -> p i h", p=128
                        ),
                        ob[:].rearrange("p (i h) -> p i h", h=64),
                    )

                for j in range(NT):
                    s0 = 128 * j
                    span = T - s0
                    wT = wtpool.tile(
                        [128, T], F16, tag="wT", bufs=4, name=f"wT{j}"
                    )
                    wTs[j] = wT
                    off = 0
                    while off < span:
                        w = min(1024, span - off)
                        sc = ps3.tile(
                            [128, 1024], F32, tag="sc", bufs=2,
                            name=f"sc{j}_{off}",
                        )
                        o2 = 0
                        while o2 < w:
                            n = min(512, w - o2)
                            t0 = s0 + off + o2
                            nc.tensor.matmul(
                                sc[0:64, o2 : o2 + n],
                                k16[:, s0 : s0 + 64],
                                q16[:, t0 : t0 + n],
                                start=True, stop=True,
                                tile_position=(0, 0), skip_group_check=True,
                            )
                            nc.tensor.matmul(
                                sc[64:128, o2 : o2 + n],
                                k16[:, s0 + 64 : s0 + 128],
                                q16[:, t0 : t0 + n],
                                start=True, stop=True,
                                tile_position=(0, 64), skip_group_check=True,
                            )
                            o2 += n
                        nc.scalar.activation(
                            wT[:, off : off + w], sc[:, 0:w],
                            mybir.ActivationFunctionType.Exp,
                            bias=zb[:, 0:1], scale=SCALE,
                        )
                        if off == 0:
                            nc.vector.tensor_mul(
                                wT[:, 0:128], wT[:, 0:128], tri[:]
                            )
                        off += w
                    if j <= 1:
                        g = j
                        tp = ps3.tile(
                            [128, 1024], F32, tag="sc", bufs=2, name=f"tp{g}"
                        )
                        for i in range(8):
                            blk = 8 * g + i
                            nc.tensor.matmul(
                                tp[:, 64 * i : 64 * i + 64],
                                vTT[64:128, 128 * blk : 128 * blk + 128],
                                eye64v[64:128, :],
                                start=True, stop=True,
                            )
                        nc.vector.tensor_copy(
                            v1[:, 520 * g : 520 * g + 520].rearrange(
                                "p (i c) -> p i c", c=65
                            )[:, :, 0:64],
                            tp[:, 0:512].rearrange("p (i c) -> p i c", c=64),
                        )
                    if j >= 1:
                        emit_pv(j - 1)
                        if (j - 1) % 4 == 3:
                            emit_finish((j - 1) // 4)
                emit_pv(NT - 1)
                emit_finish(NC - 1)

    if split_waits:
        _split_excess_waits(nc)
    return nc


_NC_CACHE = None


def _get_nc() -> bass.Bass:
    global _NC_CACHE
    if _NC_CACHE is None:
        _NC_CACHE = build_nc()
    return _NC_CACHE


def _pack_pairs(arr, m):
    """[E, m] -> [128, NG*2*m] with E = 256g + 2p + k on (partition p, pair k)."""
    return np.ascontiguousarray(
        arr.reshape(NG, 128, 2, m).transpose(1, 0, 2, 3).reshape(128, NG * 2 * m)
    )


def kernel(x, Wq, Wk, Wv, **run_kwargs):
    nc = _get_nc()
    x = np.asarray(x)
    wqv_full = np.concatenate(
        [np.asarray(Wq), np.asarray(Wv)], axis=1
    ).astype(np.float16)
    wqv_sw = np.ascontiguousarray(
        wqv_full.reshape(NE, 128, 128).transpose(1, 0, 2).reshape(128, NE * 128)
    )
    wk_sw = _pack_pairs(np.asarray(Wk).astype(NP8), H)
    eye64v = np.concatenate(
        [np.zeros((64, 64), np.float16), np.eye(64, dtype=np.float16)], axis=0
    )
    eye32 = np.eye(128, dtype=np.float32)
    tri = np.triu(np.ones((128, 128), dtype=np.float16))
    in_maps = []
    for b in range(B):
        xtb = np.ascontiguousarray(x[b].T)
        in_maps.append({
            "x16": xtb.astype(np.float16),
            "x8": _pack_pairs(xtb.astype(NP8), T),
            "wqv16": wqv_sw,
            "wk8": wk_sw,
            "eye64v": eye64v,
            "eye32": eye32,
            "tri": tri,
        })
    res = run_bass_kernel_spmd(nc, in_maps, core_ids=list(range(B)), **run_kwargs)
    out = np.stack([res.results[b]["out"] for b in range(B)], axis=0)
    kernel.last_results = res
    return out


# revision 19
# speedup vs baseline: 1.0159x; 1.0159x over previous
"""Single-head causal attention (B=8, T=2048, E=1024, H=64) on 8 trn2
cores, data-parallel over batch (core b handles batch element b).

Precision layout (validated vs gate rel_err < 2e-2; measures 8.1e-3):
  - q and v projected in fp16 via one shared stationary [Wq|Wv] (the v
    projection rides free in the 128-wide weight block),
  - k projected in fp8e4m3 DoubleRow (host pair-packs x and Wk along the
    contraction dim: E = 256g + 2p + k on partition p, pair k -> 256-deep
    contraction per matmul, half the matmul count),
  - scoresT, exp and PV all fp16 (fp8 there is LDWEIGHTS-bound and the
    v path cannot tolerate fp8 quantization at the max-error tail).

Host marshaling: x is cast/transposed/packed on host (fp16 xT + fp8
pair-packed copy), weights pre-swizzled partition-major, identity and
causal-mask constants shipped as inputs so the device has no const loads.

Device schedule per core:
  proj (overlapped with pipelined x DMAs; qv j-outer / k g-outer into all
  8 psum banks so the PE starts on the first arriving tile):
    pqv[c] += wqv_j^T @ x16_j[:, chunk]      (fp16)
    pk[c]  += wk8_g^T @ x8_g[:, :, chunk]    (fp8 DoubleRow)
    evacuate q,k -> fp16 partitions 0:63 (directly usable by scores),
    v -> rows 64:127; v1[s, 65] = PE-transpose(v) + ones column (the
    ones column makes PV accumulate the softmax denominator for free)
  attention (j = key block 0..15, PV lagging one block):
    scoresT[s,t] = k16_j^T @ q16 -> exp((1/8)x) on ACT -> wT_j fp16
    diag tri-mask on DVE; outT[65,t] += v1_j^T @ wT_j (psum bank/512-chunk)
    as chunk c completes: PE-transpose outT, reciprocal-normalize
    (DVE/ACT split), store f32
"""

import numpy as np
import ml_dtypes

import concourse.bass as bass
import concourse.mybir as mybir
from concourse.tile import TileContext
from concourse.bass_utils import run_bass_kernel_spmd

B, T, E, H = 8, 2048, 1024, 64
NT = T // 128   # 16 key/row blocks
NE = E // 128   # 8 fp16 contraction blocks
NG = E // 256   # 4 fp8 pair-contraction groups
NC = T // 512   # 4 column chunks
F16 = mybir.dt.float16
F32 = mybir.dt.float32
F8 = mybir.dt.float8e4
NP8 = ml_dtypes.float8_e4m3
SCALE = float(H) ** -0.5
DR = mybir.MatmulPerfMode.DoubleRow


def _split_excess_waits(nc: bass.Bass, cap: int = 1) -> int:
    n_split = 0
    for f in nc.m.functions:
        for bb in f.blocks:
            insts = list(bb.instructions)
            out = []
            dirty = False
            for inst in insts:
                si = inst.sync_info
                waits = list(si.on_wait) if si and si.on_wait else []
                if len(waits) > cap:
                    si.on_wait = waits[:cap]
                    for w in waits[cap:]:
                        nop = mybir.InstNoOp(
                            name=f"I-waitsplit-{n_split}", ins=[], outs=[]
                        )
                        nop.engine = inst.engine
                        nop.sync_info = mybir.SyncInfo(on_wait=[w], on_update=[])
                        out.append(nop)
                        n_split += 1
                    dirty = True
                out.append(inst)
            if dirty:
                bb.instructions = out
    return n_split


def build_nc(split_waits: bool = True) -> bass.Bass:
    nc = bass.Bass()
    x16 = nc.dram_tensor("x16", [E, T], F16, kind="ExternalInput")
    x8 = nc.dram_tensor("x8", [128, NG * 2 * T], F8, kind="ExternalInput")
    wqv16 = nc.dram_tensor("wqv16", [128, NE * 128], F16, kind="ExternalInput")
    wk8 = nc.dram_tensor("wk8", [128, NG * 2 * H], F8, kind="ExternalInput")
    eye64v_d = nc.dram_tensor("eye64v", [128, 64], F16, kind="ExternalInput")
    eye32_d = nc.dram_tensor("eye32", [128, 128], F32, kind="ExternalInput")
    tri_d = nc.dram_tensor("tri", [128, 128], F16, kind="ExternalInput")
    out = nc.dram_tensor("out", [T, H], F32, kind="ExternalOutput")
    x16_ap, x8_ap, out_ap = x16.ap(), x8.ap(), out.ap()

    with TileContext(nc) as tc:
        with (
            tc.tile_pool(name="const", bufs=1) as cpool,
            tc.tile_pool(name="wts", bufs=1) as wpool,
            tc.tile_pool(name="xt", bufs=8) as xtpool,
            tc.tile_pool(name="x8t", bufs=4) as x8pool,
            tc.tile_pool(name="qkv", bufs=1) as qkvpool,
            tc.tile_pool(name="wTp", bufs=4) as wtpool,
            tc.tile_pool(name="fin", bufs=2) as finpool,
        ):
            wqv_t = wpool.tile([128, NE * 128], F16, tag="wqv")
            nc.scalar.dma_start(wqv_t[:], wqv16.ap())
            wk_t = wpool.tile([128, NG * 2 * H], F8, tag="wk")
            nc.scalar.dma_start(wk_t[:], wk8.ap())

            xts = [
                xtpool.tile([128, T], F16, tag="xt", name=f"xt{j}")
                for j in range(NE)
            ]
            x8ts = [
                x8pool.tile([128, 2 * T], F8, tag="x8t", name=f"x8t{g}")
                for g in range(NG)
            ]
            x8_r = x8_ap.rearrange("p (g k t) -> p g k t", g=NG, k=2)
            # interleave x16/x8 loads in MM consumption order
            for j in range(NE):
                for h in range(2):
                    sl = slice(1024 * h, 1024 * h + 1024)
                    nc.sync.dma_start(
                        xts[j][:, sl], x16_ap[128 * j : 128 * j + 128, sl]
                    )
                if j % 2 == 1:
                    g = j // 2
                    xg = x8ts[g][:].rearrange("p (k t) -> p k t", k=2)
                    for h in range(2):
                        sl = slice(1024 * h, 1024 * h + 1024)
                        nc.sync.dma_start(xg[:, :, sl], x8_r[:, g, :, sl])

            eye64v = cpool.tile([128, 64], F16, tag="eye64v")
            nc.sync.dma_start(eye64v[:], eye64v_d.ap())
            eye32 = cpool.tile([128, 128], F32, tag="eye32")
            nc.sync.dma_start(eye32[:], eye32_d.ap())
            tri = cpool.tile([128, 128], F16, tag="tri")
            nc.sync.dma_start(tri[:], tri_d.ap())
            zb = cpool.tile([128, 1], F32, tag="zb")
            nc.gpsimd.memset(zb[:], 0.0)
            expwarm = cpool.tile([128, 1], F16, tag="expwarm")
            nc.scalar.activation(
                expwarm[:], zb[:], mybir.ActivationFunctionType.Exp,
                bias=zb[:, 0:1], scale=1.0,
            )

            q16 = qkvpool.tile([64, T], F16, tag="q16")
            k16 = qkvpool.tile([64, T], F16, tag="k16")
            vTT = qkvpool.tile([128, T], F16, tag="vTT")
            v1a = qkvpool.tile([128, NT * 33], F16, tag="v1a")
            v1b = qkvpool.tile([128, NT * 32], F16, tag="v1b")
            nc.gpsimd.memset(
                v1a[:].rearrange("p (i c) -> p i c", c=33)[:, :, 32:33], 1.0
            )

            wk_r = wk_t[:].rearrange("p (g k m) -> p g k m", g=NG, k=2)

            # ---------- projections ----------
            with tc.tile_pool(name="ps12", bufs=1, space="PSUM") as ps12:
                pqv = ps12.tile([128, 2048], F32, tag="pqv", bufs=1)
                pk = ps12.tile([64, 2048], F32, tag="pk", bufs=1)
                # pqv chunks (fp16, q rows 0:64 / v rows 64:128); pk (fp8 DR)
                for j in range(NE):
                    for half in range(2):
                        for c in (2 * half, 2 * half + 1):
                            nc.tensor.matmul(
                                pqv[:, 512 * c : 512 * c + 512],
                                wqv_t[:, 128 * j : 128 * j + 128],
                                xts[j][:, 512 * c : 512 * c + 512],
                                start=(j == 0), stop=(j == NE - 1),
                            )
                    if j % 2 == 1:
                        g = j // 2
                        xg = x8ts[g][:].rearrange("p (k t) -> p k t", k=2)
                        for half in range(2):
                            for c in (2 * half, 2 * half + 1):
                                nc.tensor.matmul(
                                    pk[:, 512 * c : 512 * c + 512],
                                    wk_r[:, g],
                                    xg[:, :, 512 * c : 512 * c + 512],
                                    start=(g == 0), stop=(g == NG - 1),
                                    perf_mode=DR,
                                )

                # evacuate: q,k -> fp16 at partitions 0:63, v -> rows 64:127
                # (narrow first slices so scores j=0 starts ASAP)
                s0a, s0b = slice(0, 512), slice(512, 2048)
                nc.vector.tensor_copy(q16[:, s0a], pqv[0:64, s0a])
                nc.scalar.copy(k16[:, s0a], pk[:, s0a])
                nc.vector.tensor_copy(q16[:, s0b], pqv[0:64, s0b])
                nc.scalar.copy(k16[:, s0b], pk[:, s0b])
                nc.vector.tensor_copy(
                    vTT[64:128, 0:1024], pqv[64:128, 0:1024]
                )
                nc.scalar.copy(vTT[64:128, 1024:2048], pqv[64:128, 1024:2048])

                # v1 = transpose(v) blocks + ones column
                for g in range(2):
                    tp = ps12.tile(
                        [128, 512], F32, tag="pp", bufs=8, name=f"tp{g}"
                    )
                    for i in range(8):
                        blk = 8 * g + i
                        nc.tensor.matmul(
                            tp[:, 64 * i : 64 * i + 64],
                            vTT[64:128, 128 * blk : 128 * blk + 128],
                            eye64v[64:128, :],
                            start=True, stop=True,
                        )
                    tp_r = tp[:].rearrange("p (i c) -> p i c", c=64)
                    nc.vector.tensor_copy(
                        v1a[:, 264 * g : 264 * g + 264].rearrange(
                            "p (i c) -> p i c", c=33
                        )[:, :, 0:32],
                        tp_r[:, :, 0:32],
                    )
                    nc.vector.tensor_copy(
                        v1b[:, 256 * g : 256 * g + 256].rearrange(
                            "p (i c) -> p i c", c=32
                        ),
                        tp_r[:, :, 32:64],
                    )

            # ---------- attention: scores, exp, PV, finish ----------
            with tc.tile_pool(name="ps3", bufs=1, space="PSUM") as ps3:
                otps = [
                    ps3.tile([128, 512], F32, tag="ot", bufs=4, name=f"ot{c}")
                    for c in range(NC)
                ]
                wTs = {}

                def emit_pv(j):
                    s0 = 128 * j
                    for c in range(s0 // 512, NC):
                        t0 = max(512 * c, s0)
                        t1 = 512 * c + 512
                        wsl = wTs[j][:, t0 - s0 : t1 - s0]
                        nc.tensor.matmul(
                            otps[c][0:33, t0 - 512 * c : 512],
                            v1a[:, 33 * j : 33 * j + 33],
                            wsl,
                            start=(j == 0), stop=(j == 4 * c + 3),
                            tile_position=(0, 0), skip_group_check=True,
                        )
                        nc.tensor.matmul(
                            otps[c][64:96, t0 - 512 * c : 512],
                            v1b[:, 32 * j : 32 * j + 32],
                            wsl,
                            start=(j == 0), stop=(j == 4 * c + 3),
                            tile_position=(0, 64), skip_group_check=True,
                        )

                def emit_finish(c):
                    oc = finpool.tile(
                        [128, 512], F32, tag="oc", bufs=2, name=f"oc{c}"
                    )
                    nc.vector.tensor_copy(oc[0:33, :], otps[c][0:33, :])
                    nc.scalar.copy(oc[64:96, :], otps[c][64:96, :])
                    ft = ps3.tile(
                        [128, 260], F32, tag="ot", bufs=4, name=f"ft{c}"
                    )
                    for i in range(4):
                        nc.tensor.transpose(
                            ft[:, 65 * i : 65 * i + 33],
                            oc[0:33, 128 * i : 128 * i + 128],
                            eye32[0:33, 0:33],
                        )
                        nc.tensor.transpose(
                            ft[:, 65 * i + 33 : 65 * i + 65],
                            oc[64:96, 128 * i : 128 * i + 128],
                            eye32[64:96, 64:96],
                        )
                    rcp = finpool.tile(
                        [128, 4], F32, tag="rcp", bufs=2, name=f"rcp{c}"
                    )
                    nc.vector.reciprocal(
                        rcp[:],
                        ft[:].rearrange("p (i c) -> p i c", c=65)[:, :, 32:33],
                    )
                    ob = finpool.tile(
                        [128, 256], F32, tag="ob", bufs=2, name=f"ob{c}"
                    )
                    for i in range(4):
                        if i % 2 == 0:
                            nc.vector.tensor_scalar_mul(
                                ob[:, 64 * i : 64 * i + 32],
                                ft[:, 65 * i : 65 * i + 32],
                                rcp[:, i : i + 1],
                            )
                            nc.scalar.mul(
                                ob[:, 64 * i + 32 : 64 * i + 64],
                                ft[:, 65 * i + 33 : 65 * i + 65],
                                rcp[:, i : i + 1],
                            )
                        else:
                            nc.scalar.mul(
                                ob[:, 64 * i : 64 * i + 32],
                                ft[:, 65 * i : 65 * i + 32],
                                rcp[:, i : i + 1],
                            )
                            nc.vector.tensor_scalar_mul(
                                ob[:, 64 * i + 32 : 64 * i + 64],
                                ft[:, 65 * i + 33 : 65 * i + 65],
                                rcp[:, i : i + 1],
                            )
                    nc.sync.dma_start(
                        out_ap[512 * c : 512 * c + 512, :].rearrange(
                            "(i p) h -> p i h", p=128
                        ),
                        ob[:].rearrange("p (i h) -> p i h", h=64),
                    )

                for j in range(NT):
                    s0 = 128 * j
                    span = T - s0
                    wT = wtpool.tile(
                        [128, T], F16, tag="wT", bufs=4, name=f"wT{j}"
                    )
                    wTs[j] = wT
                    off = 0
                    while off < span:
                        w = min(1024, span - off)
                        sc = ps3.tile(
                            [128, 1024], F32, tag="sc", bufs=2,
                            name=f"sc{j}_{off}",
                        )
                        o2 = 0
                        while o2 < w:
                            n = min(512, w - o2)
                            t0 = s0 + off + o2
                            nc.tensor.matmul(
                                sc[0:64, o2 : o2 + n],
                                k16[:, s0 : s0 + 64],
                                q16[:, t0 : t0 + n],
                                start=True, stop=True,
                                tile_position=(0, 0), skip_group_check=True,
                            )
                            nc.tensor.matmul(
                                sc[64:128, o2 : o2 + n],
                                k16[:, s0 + 64 : s0 + 128],
                                q16[:, t0 : t0 + n],
                                start=True, stop=True,
                                tile_position=(0, 64), skip_group_check=True,
                            )
                            o2 += n
                        nc.scalar.activation(
                            wT[:, off : off + w], sc[:, 0:w],
                            mybir.ActivationFunctionType.Exp,
                            bias=zb[:, 0:1], scale=SCALE,
                        )
                        if off == 0:
                            nc.vector.tensor_mul(
                                wT[:, 0:128], wT[:, 0:128], tri[:]
                            )
                        off += w
                    if j <= 1:
                        g = j
                        tp = ps3.tile(
                            [128, 1024], F32, tag="sc", bufs=2, name=f"tp{g}"
                        )
                        for i in range(8):
                            blk = 8 * g + i
                            nc.tensor.matmul(
                                tp[:, 64 * i : 64 * i + 64],
                                vTT[64:128, 128 * blk : 128 * blk + 128],
                                eye64v[64:128, :],
                                start=True, stop=True,
                            )
                        nc.vector.tensor_copy(
                            v1[:, 520 * g : 520 * g + 520].rearrange(
                                "p (i c) -> p i c", c=65
                            )[:, :, 0:64],
                            tp[:, 0:512].rearrange("p (i c) -> p i c", c=64),
                        )
                    if j >= 1:
                        emit_pv(j - 1)
                        if (j - 1) % 4 == 3:
                            emit_finish((j - 1) // 4)
                emit_pv(NT - 1)
                emit_finish(NC - 1)

    if split_waits:
        _split_excess_waits(nc)
    return nc


_NC_CACHE = None


def _get_nc() -> bass.Bass:
    global _NC_CACHE
    if _NC_CACHE is None:
        _NC_CACHE = build_nc()
    return _NC_CACHE


def _pack_pairs(arr, m):
    """[E, m] -> [128, NG*2*m] with E = 256g + 2p + k on (partition p, pair k)."""
    return np.ascontiguousarray(
        arr.reshape(NG, 128, 2, m).transpose(1, 0, 2, 3).reshape(128, NG * 2 * m)
    )


def kernel(x, Wq, Wk, Wv, **run_kwargs):
    nc = _get_nc()
    x = np.asarray(x)
    wqv_full = np.concatenate(
        [np.asarray(Wq), np.asarray(Wv)], axis=1
    ).astype(np.float16)
    wqv_sw = np.ascontiguousarray(
        wqv_full.reshape(NE, 128, 128).transpose(1, 0, 2).reshape(128, NE * 128)
    )
    wk_sw = _pack_pairs(np.asarray(Wk).astype(NP8), H)
    eye64v = np.concatenate(
        [np.zeros((64, 64), np.float16), np.eye(64, dtype=np.float16)], axis=0
    )
    eye32 = np.eye(128, dtype=np.float32)
    tri = np.triu(np.ones((128, 128), dtype=np.float16))
    in_maps = []
    for b in range(B):
        xtb = np.ascontiguousarray(x[b].T)
        in_maps.append({
            "x16": xtb.astype(np.float16),
            "x8": _pack_pairs(xtb.astype(NP8), T),
            "wqv16": wqv_sw,
            "wk8": wk_sw,
            "eye64v": eye64v,
            "eye32": eye32,
            "tri": tri,
        })
    res = run_bass_kernel_spmd(nc, in_maps, core_ids=list(range(B)), **run_kwargs)
    out = np.stack([res.results[b]["out"] for b in range(B)], axis=0)
    kernel.last_results = res
    return out
